# revision 44
# baseline (speedup 1.0000x reference)
"""BioLinearAttention (ELU+1 linear attention) on 8 TRN2 NeuronCores.

Sharding: token-parallel. The (B, T) = (4, 4096) grid flattens to 16384 rows;
each core owns 2048 contiguous rows (core c holds batch c//2's half). Each core
computes k/v projections for its rows, accumulates partial kv = k'^T v and
k_sum directly in PSUM across all 16 token tiles, then a pairwise AllReduce
(cores 2b, 2b+1 share batch b) completes the per-batch kv / k_sum. The q
projection for all rows runs while the collective is in flight. Stage C
computes den = q'.k_sum via block-diagonal 2-head matmuls, pre-scales
q~ = q' / den on the DVE, then y^T = kv^T_blockdiag @ q~ (K=128 2-head packed)
and the output projection.

All matmul operands are bf16 (inputs cast host-side); PSUM accumulation stays
fp32. The output is written bf16 and cast back to fp32 on host. DVE element
wise ops run on bf16 SBUF tiles to hit the 2x/4x DVE fast paths; reciprocal
stays fp32 (reciprocal_approx_fast requires it).
"""

import sys
import types

import numpy as np

B, T, C = 4, 4096, 1024
H, D = 16, 64
N_CORES = 8
ROWS = B * T
RPC = ROWS // N_CORES  # rows per core: 2048
NT = RPC // 128  # 128-token tiles per core: 16
NST = RPC // 512  # 512-token super-tiles per core: 4

_CACHE = {}


def _ensure_hook_shim():
    """bass_utils imports antenv.axon_hooks when BASS_TRACE is set; the image
    lacks that module. Provide a no-op shim unless one is already installed."""
    if "antenv.axon_hooks" in sys.modules:
        return
    try:
        import antenv
    except ImportError:
        return
    mod = types.ModuleType("antenv.axon_hooks")
    mod._hook = None
    mod.set_axon_ntff_profile_hook = lambda h: setattr(mod, "_hook", h)
    mod.get_axon_ntff_profile_hook = lambda: mod._hook
    sys.modules["antenv.axon_hooks"] = mod
    antenv.axon_hooks = mod


def _build(with_bias):
    key = ("nc", with_bias)
    if key in _CACHE:
        return _CACHE[key]

    import concourse.bacc as bacc
    import concourse.mybir as mybir
    from concourse.tile import TileContext

    F32 = mybir.dt.float32
    BF16 = mybir.dt.bfloat16
    AF = mybir.ActivationFunctionType

    nc = bacc.Bacc("TRN2", num_devices=N_CORES, debug=False)

    xt = nc.dram_tensor("xt", [C, RPC], BF16, kind="ExternalInput")
    wkvt = nc.dram_tensor("wkvt", [C, 2 * C], BF16, kind="ExternalInput")
    wqt = nc.dram_tensor("wqt", [C, C], BF16, kind="ExternalInput")
    wct = nc.dram_tensor("wct", [C, C], BF16, kind="ExternalInput")
    if with_bias:
        bkv = nc.dram_tensor("bkv", [1, 2 * C], BF16, kind="ExternalInput")
        bq = nc.dram_tensor("bq", [1, C], BF16, kind="ExternalInput")
        bc = nc.dram_tensor("bc", [1, C], BF16, kind="ExternalInput")
    out = nc.dram_tensor("out", [RPC, C], BF16, kind="ExternalOutput")
    # kv (rows 0:64) | k_sum (row 64) partials, layout [d, (h, e)]
    cc_in = nc.dram_tensor("cc_in", [D + 1, C], BF16, kind="Internal")
    cc_out = nc.dram_tensor("cc_out", [D + 1, C], BF16, kind="Internal")
    groups = [[0, 1], [2, 3], [4, 5], [6, 7]]

    with TileContext(nc) as tc:
        with (
            tc.tile_pool(name="const", bufs=1) as cst,
            tc.tile_pool(name="wts", bufs=1) as wtp,
            tc.tile_pool(name="xres", bufs=1) as xrp,
            tc.tile_pool(name="qres", bufs=1) as qrp,
            tc.tile_pool(name="kvres", bufs=1) as kvp,
        ):
            # ---- persistent SBUF ----
            x_sb = xrp.tile([128, 8, RPC], BF16)  # x^T, c-chunk major
            wkv_sb = wtp.tile([128, 8, 2 * C], BF16)
            wq_sb = wtp.tile([128, 8, C], BF16)
            wc_sb = wtp.tile([128, 8, C], BF16)
            qn_sb = qrp.tile([128, 8, RPC], BF16)  # q' (ELU+1), later q~ in place
            kv2 = kvp.tile([128, 8, 128], BF16)  # block-diag per head pair
            krep2 = kvp.tile([128, 8, 128], BF16)  # block-diag ksum-replicated
            kvt_sb = kvp.tile([D + 1, 8, 128], BF16)  # collective result
            ksb = kvp.tile([1, 8, 128], BF16)  # ksum row at partition 0
            onecol = cst.tile([128, 1], BF16)
            ones_row = cst.tile([1, 512], BF16)
            # warmup operand; memset first so the PE can start immediately
            # after engine init
            dum_sb = cst.tile([128, 512], BF16)
            nc.vector.memset(dum_sb[:], 0.0)
            if with_bias:
                bkv_sb = cst.tile([1, 2 * C], BF16)
                bq_sb = cst.tile([1, C], BF16)
                bc_sb = cst.tile([1, C], BF16)
                nc.sync.dma_start(bkv_sb[:], bkv.ap())
                nc.sync.dma_start(bq_sb[:], bq.ap())
                nc.sync.dma_start(bc_sb[:], bc.ap())

            x_re = xt.ap().rearrange("(c p) t -> p c t", p=128)
            wkv_re = wkvt.ap().rearrange("(c p) n -> p c n", p=128)

            # per-ic granularity for tile 0's weight deps so the first k-proj
            # matmul can start as soon as its slice lands; x in 256-token
            # blocks (512B contiguous runs - full DMA efficiency)
            nc.sync.dma_start(x_sb[:, :, 0:256], x_re[:, :, 0:256])
            for ic in range(8):
                nc.sync.dma_start(
                    wkv_sb[:, ic, 0:512], wkv_re[:, ic, 0:512]
                )
            nc.sync.dma_start(x_sb[:, :, 256:512], x_re[:, :, 256:512])
            for ic in range(8):
                nc.sync.dma_start(
                    wkv_sb[:, ic, 512:1024], wkv_re[:, ic, 512:1024]
                )
            for nh in range(4, 8):  # v weights in 512KB pieces
                nc.sync.dma_start(
                    wkv_sb[:, 0:4, nh * 256 : (nh + 1) * 256],
                    wkv_re[:, 0:4, nh * 256 : (nh + 1) * 256],
                )
                nc.sync.dma_start(
                    wkv_sb[:, 4:8, nh * 256 : (nh + 1) * 256],
                    wkv_re[:, 4:8, nh * 256 : (nh + 1) * 256],
                )
            nc.vector.memset(onecol[:], 1.0)
            nc.vector.memset(ones_row[:], 1.0)
            nc.vector.memset(kv2[:], 0.0)
            nc.vector.memset(krep2[:], 0.0)

            # ---------------- stage A: k/v proj, kv & k_sum PSUM accumulation
            with (
                tc.tile_pool(name="kv1", bufs=2) as kvp1,
                tc.tile_pool(name="el1", bufs=2) as el1,
                tc.tile_pool(name="ps1", bufs=6, space="PSUM") as ps1,
                tc.tile_pool(name="kvacc", bufs=1, space="PSUM") as kvap,
            ):
                kvacc = kvap.tile([D + 1, C], F32)

                # PE warmup: dummy matmuls on memset-resident tiles (no DMA
                # deps). The PE clock needs ~3us of continuous execution to
                # reach 2.4GHz; these ramp it and keep it busy while the
                # first x/weight DMAs land, so real matmuls start at full
                # clock instead of the 0.65GHz cold p-state.
                def emit_dummy(n):
                    for _ in range(n):
                        dmy = ps1.tile([128, 512], F32, tag="pk")
                        nc.tensor.matmul(
                            dmy[0:1, :],
                            lhsT=dum_sb[:, 0:1],
                            rhs=dum_sb[:],
                            start=True,
                            stop=True,
                            skip_group_check=True,
                        )

                emit_dummy(12)

                def emit_kv_outer(kq0, vq0, t0):
                    st_acc = t0 == 0
                    sp_acc = t0 == NT - 1
                    for h in range(H):
                        # start=True marks the whole 2KB PSUM bank row
                        # pending-zero, so only the first head touching
                        # each bank may set it; later heads' first-tile
                        # writes land on pending bytes and init cleanly.
                        nc.tensor.matmul(
                            kvacc[0:D, h * D : (h + 1) * D],
                            lhsT=kq0[:, h * D : (h + 1) * D],
                            rhs=vq0[:, h * D : (h + 1) * D],
                            start=st_acc and h % 8 == 0,
                            stop=sp_acc,
                            skip_group_check=True,
                        )
                    for nh in range(2):
                        nc.tensor.matmul(
                            kvacc[D : D + 1, nh * 512 : (nh + 1) * 512],
                            lhsT=onecol[:],
                            rhs=kq0[:, nh * 512 : (nh + 1) * 512],
                            start=st_acc,
                            stop=sp_acc,
                            skip_group_check=True,
                        )

                def emit_proj_chunk(tt, half, nh, kq, vq):
                    xtile = x_sb[:, :, tt * 128 : (tt + 1) * 128]
                    sl = slice(half * C + nh * 512, half * C + (nh + 1) * 512)
                    pk = ps1.tile([128, 512], F32, tag="pk")
                    for ic in range(8):
                        nc.tensor.matmul(
                            pk[:],
                            lhsT=xtile[:, ic, :],
                            rhs=wkv_sb[:, ic, sl],
                            start=(ic == 0),
                            stop=(ic == 7 and not with_bias),
                        )
                    if with_bias:
                        nc.tensor.matmul(
                            pk[:],
                            lhsT=ones_row[0:1, 0:128],
                            rhs=bkv_sb[0:1, sl],
                            start=False,
                            stop=True,
                        )
                    osl = slice(nh * 512, (nh + 1) * 512)
                    if half == 0:
                        # elu(x)+1 = relu(x) + exp(min(x, 0))
                        krelu = el1.tile([128, 512], BF16, tag="krelu")
                        nc.scalar.activation(krelu[:], pk[:], AF.Relu)
                        kmin = el1.tile([128, 512], BF16, tag="kmin")
                        nc.vector.tensor_scalar_min(kmin[:], pk[:], 0.0)
                        nc.scalar.activation(kmin[:], kmin[:], AF.Exp)
                        nc.vector.tensor_add(kq[:, osl], krelu[:], kmin[:])
                    else:
                        nc.scalar.copy(vq[:, osl], pk[:])

                # tiles 0-3 k-first: their v-chunks would otherwise block the
                # in-order PE queue on the v-weight DMA while later tiles'
                # k-work (whose weights are already resident) could run
                NSPECIAL = 4
                kqv0 = [
                    (kvp1.tile([128, C], BF16, name=f"kq{t}", tag=f"kq{t}", bufs=1),
                     kvp1.tile([128, C], BF16, name=f"vq{t}", tag=f"vq{t}", bufs=1))
                    for t in range(NSPECIAL)
                ]
                for nh in range(2):
                    for tt in range(NSPECIAL):
                        emit_proj_chunk(tt, 0, nh, *kqv0[tt])
                for tt in range(NSPECIAL):
                    emit_proj_chunk(tt, 1, 0, *kqv0[tt])
                for tt in range(NSPECIAL):
                    emit_proj_chunk(tt, 1, 1, *kqv0[tt])
                    emit_kv_outer(kqv0[tt][0], kqv0[tt][1], tt)
                # bulk loads deferred behind the startup-critical DMAs:
                # issued from the Act queue, which reaches these points at
                # ~15-30us, after the k/v weights are through the rings
                nc.scalar.dma_start(
                    x_sb[:, :, 512:1280], x_re[:, :, 512:1280]
                )
                pending = []
                for tt in range(NSPECIAL, NT):
                    if tt == NSPECIAL + 1:
                        nc.scalar.dma_start(
                            x_sb[:, :, 1280:2048], x_re[:, :, 1280:2048]
                        )
                    if tt == NSPECIAL + 2:
                        nc.scalar.dma_start(
                            wq_sb[:],
                            wqt.ap().rearrange("(c p) n -> p c n", p=128),
                        )
                    if tt == NSPECIAL + 4:
                        nc.scalar.dma_start(
                            wc_sb[:],
                            wct.ap().rearrange("(c p) n -> p c n", p=128),
                        )
                    if pending:
                        emit_kv_outer(*pending.pop(0))
                    kq = kvp1.tile([128, C], BF16, name="kq", tag="kq")
                    vq = kvp1.tile([128, C], BF16, name="vq", tag="vq")
                    for half in range(2):
                        for nh in range(2):
                            emit_proj_chunk(tt, half, nh, kq, vq)
                    pending.append((kq, vq, tt))
                for p in pending:
                    emit_kv_outer(*p)

                # kickoff AllReduce of [kv | k_sum]
                kvs = el1.tile([D + 1, C], BF16, tag="kvs", bufs=1)
                nc.scalar.copy(kvs[:], kvacc[:])
                nc.gpsimd.dma_start(cc_in.ap(), kvs[:])
                nc.gpsimd.collective_compute(
                    "AllReduce",
                    mybir.AluOpType.add,
                    replica_groups=groups,
                    ins=[cc_in.ap().opt()],
                    outs=[cc_out.ap().opt()],
                )
                nc.gpsimd.dma_start(
                    kvt_sb[:], cc_out.ap().rearrange("p (c n) -> p c n", c=8)
                )
                # partition-0 ksum copy for the krep matmul lhsT
                nc.vector.tensor_copy(ksb[:], kvt_sb[D : D + 1, :, :])

            # ---------------- stage B: q projection (overlaps the collective)
            with (
                tc.tile_pool(name="el2", bufs=3) as el2,
                tc.tile_pool(name="zpool", bufs=3) as zp,
                tc.tile_pool(name="ytz", bufs=2) as ytzp,
                tc.tile_pool(name="osb", bufs=3) as osbp,
                tc.tile_pool(name="psq", bufs=2, space="PSUM") as psq,
                tc.tile_pool(name="psden", bufs=2, space="PSUM") as psden,
                tc.tile_pool(name="psy", bufs=2, space="PSUM") as psy,
                tc.tile_pool(name="pso", bufs=2, space="PSUM") as pso,
            ):
                def emit_assembly():
                    # ---- kv2 / krep2 assembly (waits on the collective) ----
                    # krep2 first: den(0) needs it; kv2 only gates y(0) later
                    krp = psden.tile([128, 8, 64], F32, tag="dps")
                    for h in range(H):
                        po = (h % 2) * 64
                        # one bank: only the first write per partition half may
                        # set start (bank-granular pending-zero, as above)
                        nc.tensor.matmul(
                            krp[po : po + 64, h // 2, :],
                            lhsT=ksb[0:1, h // 2, po : po + 64],
                            rhs=ones_row[0:1, 0:64],
                            start=h < 2,
                            stop=True,
                            skip_group_check=True,
                        )
                    # scatter krep/kv into zero-padded block-diagonal pairs
                    for po in range(2):  # even heads -> 0:64, odd -> 64:128
                        sl64 = slice(po * 64, po * 64 + 64)
                        nc.vector.tensor_copy(
                            krep2[sl64, :, sl64],
                            krp[sl64, :, :],
                        )
                    for po in range(2):
                        sl64 = slice(po * 64, po * 64 + 64)
                        nc.vector.tensor_copy(
                            kv2[sl64, :, sl64],
                            kvt_sb[0:D, :, po * 64 : po * 64 + 64],
                        )

                def emit_den_j(st, j):
                    tsl = slice(st * 512, (st + 1) * 512)
                    dps = psden.tile([128, 512], F32, tag="dps")
                    nc.tensor.matmul(
                        dps[:],
                        lhsT=krep2[:, j, :],
                        rhs=qn_sb[:, j, tsl],
                        start=True,
                        stop=True,
                    )
                    z32 = zp.tile([128, 512], F32, tag="z32")
                    nc.vector.reciprocal_approx_fast(z32[:], dps[:])
                    z16 = zp.tile([128, 512], BF16, tag="z16")
                    nc.scalar.copy(z16[:], z32[:])
                    nc.vector.tensor_mul(
                        qn_sb[:, j, tsl], qn_sb[:, j, tsl], z16[:]
                    )

                def emit_den(st):
                    for j in range(8):
                        emit_den_j(st, j)

                for st in range(NST):
                    if st == NST - 1:
                        # collective is long done by now; assembling here
                        # overlaps the last q-proj supertile instead of
                        # stalling den(0) at the stage-C boundary; den(0)'s
                        # per-j chains are interleaved into the oc loop below
                        # so its PSUM rotation never throttles the PE
                        emit_assembly()
                    tsl = slice(st * 512, (st + 1) * 512)
                    for oc in range(8):
                        qp = psq.tile([128, 512], F32)
                        for ic in range(8):
                            nc.tensor.matmul(
                                qp[:],
                                lhsT=wq_sb[:, ic, oc * 128 : (oc + 1) * 128],
                                rhs=x_sb[:, ic, tsl],
                                start=(ic == 0),
                                stop=(ic == 7 and not with_bias),
                            )
                        if with_bias:
                            nc.tensor.matmul(
                                qp[:],
                                lhsT=bq_sb[0:1, oc * 128 : (oc + 1) * 128],
                                rhs=ones_row[0:1, :],
                                start=False,
                                stop=True,
                            )
                        # elu(x)+1 = relu(x) + exp(min(x, 0))
                        qrelu = el2.tile([128, 512], BF16, tag="qrelu")
                        nc.scalar.activation(qrelu[:], qp[:], AF.Relu)
                        qmin = el2.tile([128, 512], BF16, tag="qmin")
                        nc.vector.tensor_scalar_min(qmin[:], qp[:], 0.0)
                        nc.scalar.activation(qmin[:], qmin[:], AF.Exp)
                        nc.vector.tensor_add(
                            qn_sb[:, oc, tsl], qrelu[:], qmin[:]
                        )
                        if st == NST - 1:
                            emit_den_j(0, oc)

                # ---- stage C: y^T = blockdiag(kv)^T q~, then c_proj; den for
                # st+1 emitted between y(st) and c_proj(st) so its recip/mul
                # chain overlaps c_proj instead of bunching on the DVE
                for st in range(NST):
                    tsl = slice(st * 512, (st + 1) * 512)
                    ytz = ytzp.tile([128, 8, 512], BF16)
                    for j in range(8):
                        yps = psy.tile([128, 512], F32)
                        nc.tensor.matmul(
                            yps[:],
                            lhsT=kv2[:, j, :],
                            rhs=qn_sb[:, j, tsl],
                            start=True,
                            stop=True,
                        )
                        # alternate Act/DVE so all 8 land in ~half the time;
                        # c_proj's first accumulation chain needs every j
                        if j % 2 == 0:
                            nc.scalar.copy(ytz[:, j, :], yps[:])
                        else:
                            nc.vector.tensor_copy(ytz[:, j, :], yps[:])
                    if st + 1 < NST:
                        emit_den(st + 1)
                    for k in range(4):
                        gt = st * 4 + k
                        for ch in range(2):
                            op2 = pso.tile([128, 512], F32)
                            for oc2 in range(8):
                                nc.tensor.matmul(
                                    op2[:],
                                    lhsT=ytz[:, oc2, k * 128 : (k + 1) * 128],
                                    rhs=wc_sb[:, oc2, ch * 512 : (ch + 1) * 512],
                                    start=(oc2 == 0),
                                    stop=(oc2 == 7 and not with_bias),
                                )
                            if with_bias:
                                nc.tensor.matmul(
                                    op2[:],
                                    lhsT=ones_row[0:1, 0:128],
                                    rhs=bc_sb[0:1, ch * 512 : (ch + 1) * 512],
                                    start=False,
                                    stop=True,
                                )
                            osb = osbp.tile([128, 512], BF16)
                            nc.scalar.copy(osb[:], op2[:])
                            # issue from the Act queue: no cross-engine sem
                            # between the copy and the DMA trigger
                            nc.scalar.dma_start(
                                out.ap()[
                                    gt * 128 : (gt + 1) * 128,
                                    ch * 512 : (ch + 1) * 512,
                                ],
                                osb[:],
                            )

    nc.compile()
    _CACHE[key] = nc
    return nc


LAST_RESULT = None


def kernel(x, Wq, bq, Wk, bk, Wv, bv, Wc, bc):
    global LAST_RESULT
    _ensure_hook_shim()
    import ml_dtypes
    from concourse.bass_utils import run_bass_kernel_spmd

    BF = ml_dtypes.bfloat16

    bq = np.asarray(bq, np.float32)
    bk = np.asarray(bk, np.float32)
    bv = np.asarray(bv, np.float32)
    bc = np.asarray(bc, np.float32)
    with_bias = bool(bq.any() or bk.any() or bv.any() or bc.any())
    nc = _build(with_bias)

    x = np.ascontiguousarray(np.asarray(x, dtype=np.float32))
    xt_full = np.ascontiguousarray(x.reshape(ROWS, C).T.astype(BF))  # [C, ROWS]
    wkvt = np.ascontiguousarray(
        np.concatenate(
            [np.asarray(Wk, np.float32).T, np.asarray(Wv, np.float32).T], axis=1
        ).astype(BF)
    )
    wqt = np.ascontiguousarray(np.asarray(Wq, np.float32).T.astype(BF))
    wct = np.ascontiguousarray(np.asarray(Wc, np.float32).T.astype(BF))

    in_maps = []
    for c in range(N_CORES):
        m = {
            "xt": np.ascontiguousarray(xt_full[:, c * RPC : (c + 1) * RPC]),
            "wkvt": wkvt,
            "wqt": wqt,
            "wct": wct,
        }
        if with_bias:
            m["bkv"] = np.concatenate([bk, bv]).reshape(1, 2 * C).astype(BF)
            m["bq"] = bq.reshape(1, C).astype(BF)
            m["bc"] = bc.reshape(1, C).astype(BF)
        in_maps.append(m)

    res = run_bass_kernel_spmd(nc, in_maps, core_ids=list(range(N_CORES)))
    LAST_RESULT = res
    out = np.concatenate(
        [
            np.asarray(res.results[c]["out"]).astype(np.float32)
            for c in range(N_CORES)
        ],
        axis=0,
    )
    return out.reshape(B, T, C)


# revision 45
# speedup vs baseline: 1.0652x; 1.0652x over previous
"""BioLinearAttention (ELU+1 linear attention) on 8 TRN2 NeuronCores.

Sharding: token-parallel. The (B, T) = (4, 4096) grid flattens to 16384 rows;
each core owns 2048 contiguous rows (core c holds batch c//2's half). Each core
computes k/v projections for its rows, accumulates partial kv = k'^T v and
k_sum directly in PSUM across all 16 token tiles, then a pairwise AllReduce
(cores 2b, 2b+1 share batch b) completes the per-batch kv / k_sum. The q
projection for all rows runs while the collective is in flight. Stage C
computes den = q'.k_sum via block-diagonal 2-head matmuls, pre-scales
q~ = q' / den on the DVE, then y^T = kv^T_blockdiag @ q~ (K=128 2-head packed)
and the output projection.

All matmul operands are bf16 (inputs cast host-side); PSUM accumulation stays
fp32. The output is written bf16 and cast back to fp32 on host. DVE element
wise ops run on bf16 SBUF tiles to hit the 2x/4x DVE fast paths; reciprocal
stays fp32 (reciprocal_approx_fast requires it).
"""

import sys
import types

import numpy as np

B, T, C = 4, 4096, 1024
H, D = 16, 64
N_CORES = 8
ROWS = B * T
RPC = ROWS // N_CORES  # rows per core: 2048
NT = RPC // 128  # 128-token tiles per core: 16
NST = RPC // 512  # 512-token super-tiles per core: 4

_CACHE = {}


def _ensure_hook_shim():
    """bass_utils imports antenv.axon_hooks when BASS_TRACE is set; the image
    lacks that module. Provide a no-op shim unless one is already installed."""
    if "antenv.axon_hooks" in sys.modules:
        return
    try:
        import antenv
    except ImportError:
        return
    mod = types.ModuleType("antenv.axon_hooks")
    mod._hook = None
    mod.set_axon_ntff_profile_hook = lambda h: setattr(mod, "_hook", h)
    mod.get_axon_ntff_profile_hook = lambda: mod._hook
    sys.modules["antenv.axon_hooks"] = mod
    antenv.axon_hooks = mod


def _build(with_bias):
    key = ("nc", with_bias)
    if key in _CACHE:
        return _CACHE[key]

    import concourse.bacc as bacc
    import concourse.mybir as mybir
    from concourse.tile import TileContext

    F32 = mybir.dt.float32
    BF16 = mybir.dt.bfloat16
    AF = mybir.ActivationFunctionType

    nc = bacc.Bacc("TRN2", num_devices=N_CORES, debug=False)

    xt = nc.dram_tensor("xt", [C, RPC], BF16, kind="ExternalInput")
    wkvt = nc.dram_tensor("wkvt", [C, 2 * C], BF16, kind="ExternalInput")
    wqt = nc.dram_tensor("wqt", [C, C], BF16, kind="ExternalInput")
    wct = nc.dram_tensor("wct", [C, C], BF16, kind="ExternalInput")
    if with_bias:
        bkv = nc.dram_tensor("bkv", [1, 2 * C], BF16, kind="ExternalInput")
        bq = nc.dram_tensor("bq", [1, C], BF16, kind="ExternalInput")
        bc = nc.dram_tensor("bc", [1, C], BF16, kind="ExternalInput")
    out = nc.dram_tensor("out", [RPC, C], BF16, kind="ExternalOutput")
    # kv (rows 0:64) | k_sum (row 64) partials, layout [d, (h, e)]
    cc_in = nc.dram_tensor("cc_in", [D + 1, C], BF16, kind="Internal")
    cc_out = nc.dram_tensor("cc_out", [D + 1, C], BF16, kind="Internal")
    groups = [[0, 1], [2, 3], [4, 5], [6, 7]]

    with TileContext(nc) as tc:
        with (
            tc.tile_pool(name="const", bufs=1) as cst,
            tc.tile_pool(name="wts", bufs=1) as wtp,
            tc.tile_pool(name="xres", bufs=1) as xrp,
            tc.tile_pool(name="qres", bufs=1) as qrp,
            tc.tile_pool(name="kvres", bufs=1) as kvp,
        ):
            # ---- persistent SBUF ----
            x_sb = xrp.tile([128, 8, RPC], BF16)  # x^T, c-chunk major
            wkv_sb = wtp.tile([128, 8, 2 * C], BF16)
            wq_sb = wtp.tile([128, 8, C], BF16)
            wc_sb = wtp.tile([128, 8, C], BF16)
            qn_sb = qrp.tile([128, 8, RPC], BF16)  # q' (ELU+1), later q~ in place
            kv2 = kvp.tile([128, 8, 128], BF16)  # block-diag per head pair
            krep2 = kvp.tile([128, 8, 128], BF16)  # block-diag ksum-replicated
            kvt_sb = kvp.tile([D + 1, 8, 128], BF16)  # collective result
            ksb = kvp.tile([1, 8, 128], BF16)  # ksum row at partition 0
            onecol = cst.tile([128, 1], BF16)
            ones_row = cst.tile([1, 512], BF16)
            # warmup operand; memset first so the PE can start immediately
            # after engine init
            dum_sb = cst.tile([128, 512], BF16)
            nc.vector.memset(dum_sb[:], 0.0)
            if with_bias:
                bkv_sb = cst.tile([1, 2 * C], BF16)
                bq_sb = cst.tile([1, C], BF16)
                bc_sb = cst.tile([1, C], BF16)
                nc.sync.dma_start(bkv_sb[:], bkv.ap())
                nc.sync.dma_start(bq_sb[:], bq.ap())
                nc.sync.dma_start(bc_sb[:], bc.ap())

            x_re = xt.ap().rearrange("(c p) t -> p c t", p=128)
            wkv_re = wkvt.ap().rearrange("(c p) n -> p c n", p=128)

            # per-ic granularity for tile 0's weight deps so the first k-proj
            # matmul can start as soon as its slice lands; x in 256-token
            # blocks (512B contiguous runs - full DMA efficiency)
            nc.sync.dma_start(x_sb[:, :, 0:256], x_re[:, :, 0:256])
            for ic in range(8):
                nc.sync.dma_start(
                    wkv_sb[:, ic, 0:512], wkv_re[:, ic, 0:512]
                )
            nc.sync.dma_start(x_sb[:, :, 256:512], x_re[:, :, 256:512])
            for ic in range(8):
                nc.sync.dma_start(
                    wkv_sb[:, ic, 512:1024], wkv_re[:, ic, 512:1024]
                )
            for nh in range(4, 8):  # v weights in 512KB pieces
                nc.sync.dma_start(
                    wkv_sb[:, 0:4, nh * 256 : (nh + 1) * 256],
                    wkv_re[:, 0:4, nh * 256 : (nh + 1) * 256],
                )
                nc.sync.dma_start(
                    wkv_sb[:, 4:8, nh * 256 : (nh + 1) * 256],
                    wkv_re[:, 4:8, nh * 256 : (nh + 1) * 256],
                )
            nc.vector.memset(onecol[:], 1.0)
            nc.vector.memset(ones_row[:], 1.0)
            nc.vector.memset(kv2[:], 0.0)
            nc.vector.memset(krep2[:], 0.0)
            for half in range(2):  # rest of x
                nc.sync.dma_start(
                    x_sb[:, :, 512 + half * 768 : 512 + (half + 1) * 768],
                    x_re[:, :, 512 + half * 768 : 512 + (half + 1) * 768],
                )
            nc.sync.dma_start(
                wq_sb[:], wqt.ap().rearrange("(c p) n -> p c n", p=128)
            )
            nc.sync.dma_start(
                wc_sb[:], wct.ap().rearrange("(c p) n -> p c n", p=128)
            )

            # ---------------- stage A: k/v proj, kv & k_sum PSUM accumulation
            with (
                tc.tile_pool(name="kv1", bufs=2) as kvp1,
                tc.tile_pool(name="el1", bufs=2) as el1,
                tc.tile_pool(name="ps1", bufs=6, space="PSUM") as ps1,
                tc.tile_pool(name="kvacc", bufs=1, space="PSUM") as kvap,
            ):
                kvacc = kvap.tile([D + 1, C], F32)

                # PE warmup: dummy matmuls on memset-resident tiles (no DMA
                # deps). The PE clock needs ~3us of continuous execution to
                # reach 2.4GHz; these ramp it and keep it busy while the
                # first x/weight DMAs land, so real matmuls start at full
                # clock instead of the 0.65GHz cold p-state.
                def emit_dummy(n):
                    for _ in range(n):
                        dmy = ps1.tile([128, 512], F32, tag="pk")
                        nc.tensor.matmul(
                            dmy[0:1, :],
                            lhsT=dum_sb[:, 0:1],
                            rhs=dum_sb[:],
                            start=True,
                            stop=True,
                            skip_group_check=True,
                        )

                emit_dummy(10)

                def emit_kv_outer(kq0, vq0, t0):
                    st_acc = t0 == 0
                    sp_acc = t0 == NT - 1
                    for h in range(H):
                        # start=True marks the whole 2KB PSUM bank row
                        # pending-zero, so only the first head touching
                        # each bank may set it; later heads' first-tile
                        # writes land on pending bytes and init cleanly.
                        nc.tensor.matmul(
                            kvacc[0:D, h * D : (h + 1) * D],
                            lhsT=kq0[:, h * D : (h + 1) * D],
                            rhs=vq0[:, h * D : (h + 1) * D],
                            start=st_acc and h % 8 == 0,
                            stop=sp_acc,
                            skip_group_check=True,
                        )
                    for nh in range(2):
                        nc.tensor.matmul(
                            kvacc[D : D + 1, nh * 512 : (nh + 1) * 512],
                            lhsT=onecol[:],
                            rhs=kq0[:, nh * 512 : (nh + 1) * 512],
                            start=st_acc,
                            stop=sp_acc,
                            skip_group_check=True,
                        )

                def emit_proj_chunk(tt, half, nh, kq, vq):
                    xtile = x_sb[:, :, tt * 128 : (tt + 1) * 128]
                    sl = slice(half * C + nh * 512, half * C + (nh + 1) * 512)
                    pk = ps1.tile([128, 512], F32, tag="pk")
                    for ic in range(8):
                        nc.tensor.matmul(
                            pk[:],
                            lhsT=xtile[:, ic, :],
                            rhs=wkv_sb[:, ic, sl],
                            start=(ic == 0),
                            stop=(ic == 7 and not with_bias),
                        )
                    if with_bias:
                        nc.tensor.matmul(
                            pk[:],
                            lhsT=ones_row[0:1, 0:128],
                            rhs=bkv_sb[0:1, sl],
                            start=False,
                            stop=True,
                        )
                    osl = slice(nh * 512, (nh + 1) * 512)
                    if half == 0:
                        # elu(x)+1 = relu(x) + exp(min(x, 0))
                        krelu = el1.tile([128, 512], BF16, tag="krelu")
                        nc.scalar.activation(krelu[:], pk[:], AF.Relu)
                        kmin = el1.tile([128, 512], BF16, tag="kmin")
                        nc.vector.tensor_scalar_min(kmin[:], pk[:], 0.0)
                        nc.scalar.activation(kmin[:], kmin[:], AF.Exp)
                        nc.vector.tensor_add(kq[:, osl], krelu[:], kmin[:])
                    else:
                        nc.scalar.copy(vq[:, osl], pk[:])

                # tiles 0-3 k-first: their v-chunks would otherwise block the
                # in-order PE queue on the v-weight DMA while later tiles'
                # k-work (whose weights are already resident) could run
                NSPECIAL = 4
                kqv0 = [
                    (kvp1.tile([128, C], BF16, name=f"kq{t}", tag=f"kq{t}", bufs=1),
                     kvp1.tile([128, C], BF16, name=f"vq{t}", tag=f"vq{t}", bufs=1))
                    for t in range(NSPECIAL)
                ]
                for nh in range(2):
                    for tt in range(NSPECIAL):
                        emit_proj_chunk(tt, 0, nh, *kqv0[tt])
                for tt in range(NSPECIAL):
                    emit_proj_chunk(tt, 1, 0, *kqv0[tt])
                for tt in range(NSPECIAL):
                    emit_proj_chunk(tt, 1, 1, *kqv0[tt])
                    emit_kv_outer(kqv0[tt][0], kqv0[tt][1], tt)
                pending = []
                for tt in range(NSPECIAL, NT):
                    if pending:
                        emit_kv_outer(*pending.pop(0))
                    kq = kvp1.tile([128, C], BF16, name="kq", tag="kq")
                    vq = kvp1.tile([128, C], BF16, name="vq", tag="vq")
                    for half in range(2):
                        for nh in range(2):
                            emit_proj_chunk(tt, half, nh, kq, vq)
                    pending.append((kq, vq, tt))
                for p in pending:
                    emit_kv_outer(*p)

                # kickoff AllReduce of [kv | k_sum]
                kvs = el1.tile([D + 1, C], BF16, tag="kvs", bufs=1)
                nc.scalar.copy(kvs[:], kvacc[:])
                nc.gpsimd.dma_start(cc_in.ap(), kvs[:])
                nc.gpsimd.collective_compute(
                    "AllReduce",
                    mybir.AluOpType.add,
                    replica_groups=groups,
                    ins=[cc_in.ap().opt()],
                    outs=[cc_out.ap().opt()],
                )
                nc.gpsimd.dma_start(
                    kvt_sb[:], cc_out.ap().rearrange("p (c n) -> p c n", c=8)
                )
                # partition-0 ksum copy for the krep matmul lhsT
                nc.vector.tensor_copy(ksb[:], kvt_sb[D : D + 1, :, :])

            # ---------------- stage B: q projection (overlaps the collective)
            with (
                tc.tile_pool(name="el2", bufs=3) as el2,
                tc.tile_pool(name="zpool", bufs=3) as zp,
                tc.tile_pool(name="ytz", bufs=2) as ytzp,
                tc.tile_pool(name="osb", bufs=3) as osbp,
                tc.tile_pool(name="psq", bufs=2, space="PSUM") as psq,
                tc.tile_pool(name="psden", bufs=2, space="PSUM") as psden,
                tc.tile_pool(name="psy", bufs=2, space="PSUM") as psy,
                tc.tile_pool(name="pso", bufs=2, space="PSUM") as pso,
            ):
                def emit_assembly():
                    # ---- kv2 / krep2 assembly (waits on the collective) ----
                    # krep2 first: den(0) needs it; kv2 only gates y(0) later
                    krp = psden.tile([128, 8, 64], F32, tag="dps")
                    for h in range(H):
                        po = (h % 2) * 64
                        # one bank: only the first write per partition half may
                        # set start (bank-granular pending-zero, as above)
                        nc.tensor.matmul(
                            krp[po : po + 64, h // 2, :],
                            lhsT=ksb[0:1, h // 2, po : po + 64],
                            rhs=ones_row[0:1, 0:64],
                            start=h < 2,
                            stop=True,
                            skip_group_check=True,
                        )
                    # scatter krep/kv into zero-padded block-diagonal pairs
                    for po in range(2):  # even heads -> 0:64, odd -> 64:128
                        sl64 = slice(po * 64, po * 64 + 64)
                        nc.vector.tensor_copy(
                            krep2[sl64, :, sl64],
                            krp[sl64, :, :],
                        )
                    for po in range(2):
                        sl64 = slice(po * 64, po * 64 + 64)
                        nc.vector.tensor_copy(
                            kv2[sl64, :, sl64],
                            kvt_sb[0:D, :, po * 64 : po * 64 + 64],
                        )

                def emit_den_j(st, j):
                    tsl = slice(st * 512, (st + 1) * 512)
                    dps = psden.tile([128, 512], F32, tag="dps")
                    nc.tensor.matmul(
                        dps[:],
                        lhsT=krep2[:, j, :],
                        rhs=qn_sb[:, j, tsl],
                        start=True,
                        stop=True,
                    )
                    z32 = zp.tile([128, 512], F32, tag="z32")
                    nc.vector.reciprocal_approx_fast(z32[:], dps[:])
                    z16 = zp.tile([128, 512], BF16, tag="z16")
                    nc.scalar.copy(z16[:], z32[:])
                    nc.vector.tensor_mul(
                        qn_sb[:, j, tsl], qn_sb[:, j, tsl], z16[:]
                    )

                def emit_den(st):
                    for j in range(8):
                        emit_den_j(st, j)

                for st in range(NST):
                    if st == NST - 1:
                        # collective is long done by now; assembling here
                        # overlaps the last q-proj supertile instead of
                        # stalling den(0) at the stage-C boundary; den(0)'s
                        # per-j chains are interleaved into the oc loop below
                        # so its PSUM rotation never throttles the PE
                        emit_assembly()
                    tsl = slice(st * 512, (st + 1) * 512)
                    for oc in range(8):
                        qp = psq.tile([128, 512], F32)
                        for ic in range(8):
                            nc.tensor.matmul(
                                qp[:],
                                lhsT=wq_sb[:, ic, oc * 128 : (oc + 1) * 128],
                                rhs=x_sb[:, ic, tsl],
                                start=(ic == 0),
                                stop=(ic == 7 and not with_bias),
                            )
                        if with_bias:
                            nc.tensor.matmul(
                                qp[:],
                                lhsT=bq_sb[0:1, oc * 128 : (oc + 1) * 128],
                                rhs=ones_row[0:1, :],
                                start=False,
                                stop=True,
                            )
                        # elu(x)+1 = relu(x) + exp(min(x, 0))
                        qrelu = el2.tile([128, 512], BF16, tag="qrelu")
                        nc.scalar.activation(qrelu[:], qp[:], AF.Relu)
                        qmin = el2.tile([128, 512], BF16, tag="qmin")
                        nc.vector.tensor_scalar_min(qmin[:], qp[:], 0.0)
                        nc.scalar.activation(qmin[:], qmin[:], AF.Exp)
                        nc.vector.tensor_add(
                            qn_sb[:, oc, tsl], qrelu[:], qmin[:]
                        )
                        if st == NST - 1:
                            emit_den_j(0, oc)

                # ---- stage C: y^T = blockdiag(kv)^T q~, then c_proj; den for
                # st+1 emitted between y(st) and c_proj(st) so its recip/mul
                # chain overlaps c_proj instead of bunching on the DVE
                for st in range(NST):
                    tsl = slice(st * 512, (st + 1) * 512)
                    ytz = ytzp.tile([128, 8, 512], BF16)
                    for j in range(8):
                        yps = psy.tile([128, 512], F32)
                        nc.tensor.matmul(
                            yps[:],
                            lhsT=kv2[:, j, :],
                            rhs=qn_sb[:, j, tsl],
                            start=True,
                            stop=True,
                        )
                        # alternate Act/DVE so all 8 land in ~half the time;
                        # c_proj's first accumulation chain needs every j
                        if j % 2 == 0:
                            nc.scalar.copy(ytz[:, j, :], yps[:])
                        else:
                            nc.vector.tensor_copy(ytz[:, j, :], yps[:])
                    if st + 1 < NST:
                        emit_den(st + 1)
                    for k in range(4):
                        gt = st * 4 + k
                        for ch in range(2):
                            op2 = pso.tile([128, 512], F32)
                            for oc2 in range(8):
                                nc.tensor.matmul(
                                    op2[:],
                                    lhsT=ytz[:, oc2, k * 128 : (k + 1) * 128],
                                    rhs=wc_sb[:, oc2, ch * 512 : (ch + 1) * 512],
                                    start=(oc2 == 0),
                                    stop=(oc2 == 7 and not with_bias),
                                )
                            if with_bias:
                                nc.tensor.matmul(
                                    op2[:],
                                    lhsT=ones_row[0:1, 0:128],
                                    rhs=bc_sb[0:1, ch * 512 : (ch + 1) * 512],
                                    start=False,
                                    stop=True,
                                )
                            osb = osbp.tile([128, 512], BF16)
                            nc.scalar.copy(osb[:], op2[:])
                            # issue from the Act queue: no cross-engine sem
                            # between the copy and the DMA trigger
                            nc.scalar.dma_start(
                                out.ap()[
                                    gt * 128 : (gt + 1) * 128,
                                    ch * 512 : (ch + 1) * 512,
                                ],
                                osb[:],
                            )

    nc.compile()
    _CACHE[key] = nc
    return nc


LAST_RESULT = None


def kernel(x, Wq, bq, Wk, bk, Wv, bv, Wc, bc):
    global LAST_RESULT
    _ensure_hook_shim()
    import ml_dtypes
    from concourse.bass_utils import run_bass_kernel_spmd

    BF = ml_dtypes.bfloat16

    bq = np.asarray(bq, np.float32)
    bk = np.asarray(bk, np.float32)
    bv = np.asarray(bv, np.float32)
    bc = np.asarray(bc, np.float32)
    with_bias = bool(bq.any() or bk.any() or bv.any() or bc.any())
    nc = _build(with_bias)

    x = np.ascontiguousarray(np.asarray(x, dtype=np.float32))
    xt_full = np.ascontiguousarray(x.reshape(ROWS, C).T.astype(BF))  # [C, ROWS]
    wkvt = np.ascontiguousarray(
        np.concatenate(
            [np.asarray(Wk, np.float32).T, np.asarray(Wv, np.float32).T], axis=1
        ).astype(BF)
    )
    wqt = np.ascontiguousarray(np.asarray(Wq, np.float32).T.astype(BF))
    wct = np.ascontiguousarray(np.asarray(Wc, np.float32).T.astype(BF))

    in_maps = []
    for c in range(N_CORES):
        m = {
            "xt": np.ascontiguousarray(xt_full[:, c * RPC : (c + 1) * RPC]),
            "wkvt": wkvt,
            "wqt": wqt,
            "wct": wct,
        }
        if with_bias:
            m["bkv"] = np.concatenate([bk, bv]).reshape(1, 2 * C).astype(BF)
            m["bq"] = bq.reshape(1, C).astype(BF)
            m["bc"] = bc.reshape(1, C).astype(BF)
        in_maps.append(m)

    res = run_bass_kernel_spmd(nc, in_maps, core_ids=list(range(N_CORES)))
    LAST_RESULT = res
    out = np.concatenate(
        [
            np.asarray(res.results[c]["out"]).astype(np.float32)
            for c in range(N_CORES)
        ],
        axis=0,
    )
    return out.reshape(B, T, C)
